# revision 48
# baseline (speedup 1.0000x reference)
"""Single-head causal attention (V=K source bug) on 8 trn2 NeuronCores.

Problem: x[4,2048,1024], W_Q/W_K/W_V[64,1024] (W_V unused by reference).
  Q = x @ W_Q.T ; K = x @ W_K.T ; V = K (reference bug)
  out = softmax(mask(Q K^T / sqrt(1024))) @ V      -> [4,2048,64]

Sharding: 2 cores per batch (core i: batch = i % 4, role r = i // 4).
Each batch's 8 query tiles of 256 rows split by parity (r=0 even, r=1 odd).
ONE SPMD graph for all 8 cores. Per-core differences are folded into DATA:

 * x^T is sent column-PERMUTED, own query tiles first:
     positions 0..3 = own tiles (2j+r), positions 4..7 = other tiles.
   So the Q projection reads compile-time columns [0,1024); causality over
   the permuted key order is encoded in per-core 0/1 masks.
 * slot j (own tile 2j+r, query rows 256 of it) attends own chunks
   [0..2j+1] and other chunks [8..8+2j+1] (uniform r=1 shape; r=0 masks
   the over-provisioned tail) -> 4j+4 key chunks of 128.

Device pipeline: warmup matmuls (HAM) -> col-paired projections
(Q pair, K stack A = cols 0-511|1024-1535, K stack B) -> PE transposes for
V natural -> per-slot: row-packed S^T pairs (own chunk on array rows 0-63,
other chunk on rows 64-127, concurrent), exp on ACT (scale folded), mask
mul on the final group, PV matmul with a ones-column producing the softmax
denominator in row 64. Host normalizes + transposes the [65,1024] output.
No collectives (latency floor >> kernel time).
"""

import os
import sys

sys.path.insert(0, "/opt/trn_rl_repo")

import numpy as np
import ml_dtypes

BF16 = ml_dtypes.bfloat16

B, T, C, D = 4, 2048, 1024, 64
N_CORES = 8
QTILE = 256          # query rows per slot
N_SLOTS = 4
CHUNK = 128          # key chunk
GROUP = 4            # chunks per exp group ([128, 4*256] psum tile)
SCALE = C ** -0.5
N_WARMUP = 80        # HAM warmup matmuls (cover the DMA wait before Q proj)

TRACE = False
TRACE_CORES = None
LAST_RESULTS = None


def _slot_groups_def(j):
    """Groups of 4 chunks for slot j with mask kind per group.
    Kinds: 'mixed' (slices 0-1 own diag MUL, 2-3 oth TS),
           'own_diag' (slices 2-3 MUL), 'oth_tail' (slices 2-3 TS),
           'plain'. Own chunks are 0..2j+1, other chunks 8..8+2j+1."""
    if j == 0:
        return [([0, 1, 8, 9], "mixed")]
    if j == 1:
        return [([0, 1, 2, 3], "own_diag"), ([8, 9, 10, 11], "oth_tail")]
    if j == 2:
        return [
            ([0, 1, 2, 3], "plain"),
            ([8, 9, 10, 11], "plain"),
            ([4, 5, 12, 13], "mixed"),
        ]
    return [
        ([0, 1, 2, 3], "plain"),
        ([8, 9, 10, 11], "plain"),
        ([4, 5, 6, 7], "own_diag"),
        ([12, 13, 14, 15], "oth_tail"),
    ]


def _chunk_stack(c):
    """abs permuted chunk c -> (stack_idx, half, within). Stack A covers
    permuted cols 0-511 (top) and 1024-1535 (bottom); B covers 512-1023
    (top) and 1536-2047 (bottom)."""
    pos = c // 2            # 256-col tile position 0..7
    if pos < 4:             # own side -> top halves
        return (pos // 2, 0, c % 4)
    else:                   # other side -> bottom halves
        return ((pos - 4) // 2, 1, c % 4)


def _build_graph():
    import concourse.bass as bass
    import concourse.mybir as mybir
    import concourse.tile as tile
    from concourse import bacc
    from concourse.masks import make_identity
    from contextlib import ExitStack

    fp32 = mybir.dt.float32
    bf16 = mybir.dt.bfloat16

    nc = bacc.Bacc(
        "TRN2",
        target_bir_lowering=False,
        debug=False,
        num_devices=N_CORES,
    )

    xkt = nc.dram_tensor("xkt", [C, T], bf16, kind="ExternalInput").ap()
    wkq = nc.dram_tensor("wkq", [C, 2 * D], bf16, kind="ExternalInput").ap()
    maskd = nc.dram_tensor(
        "mask", [CHUNK, 2 * N_SLOTS, QTILE], bf16, kind="ExternalInput"
    ).ap()
    scald = nc.dram_tensor(
        "scal", [CHUNK, N_SLOTS], fp32, kind="ExternalInput"
    ).ap()
    out = nc.dram_tensor(
        "out", [D + 1, N_SLOTS * QTILE], fp32, kind="ExternalOutput"
    ).ap()

    NQ = N_SLOTS * QTILE           # 1024 own query cols
    NCH = T // CHUNK               # 16 key chunks
    CCH = C // CHUNK               # 8 contraction chunks

    with tile.TileContext(nc) as tc, ExitStack() as ctx:
        consts = ctx.enter_context(tc.tile_pool(name="consts", bufs=1))
        xpool = ctx.enter_context(tc.tile_pool(name="xpool", bufs=1))
        kqpool = ctx.enter_context(tc.tile_pool(name="kqpool", bufs=1))
        ptpool = ctx.enter_context(tc.tile_pool(name="ptpool", bufs=10))
        opool = ctx.enter_context(tc.tile_pool(name="opool", bufs=2))
        psP = ctx.enter_context(tc.tile_pool(name="psP", bufs=2, space="PSUM"))
        psS = ctx.enter_context(tc.tile_pool(name="psS", bufs=2, space="PSUM"))
        psO = ctx.enter_context(tc.tile_pool(name="psO", bufs=2, space="PSUM"))

        # ---- constants ----
        # warmup matmuls on a memset tile: near-zero deps, start immediately
        warm_src = consts.tile([128, 128], bf16)
        nc.vector.memset(warm_src, 0.0)
        warm_ps = psP.tile([128, 128], fp32, tag="proj")
        for w in range(N_WARMUP):
            nc.tensor.matmul(
                warm_ps, lhsT=warm_src, rhs=warm_src,
                start=(w == 0), stop=(w == N_WARMUP - 1),
            )
        ident = consts.tile([128, 128], bf16)
        make_identity(nc, ident)
        warm = consts.tile([1, 1], fp32)
        nc.vector.memset(warm, 0.0)
        nc.scalar.activation(warm, warm, mybir.ActivationFunctionType.Exp)

        # ---- DMAs (slab order drives the pipeline) ----
        w_sb = consts.tile([128, CCH, 2 * D], bf16)
        nc.sync.dma_start(out=w_sb, in_=wkq.rearrange("(c p) d -> p c d", p=128))
        # xkt slabs: 4 x [128, CCH, 512] column slabs of the permuted x^T
        xs = []
        xkt_r = xkt.rearrange("(c p) t -> p c t", p=128)
        # interleave slab halves so Q (s0+s1) completes earliest, then A
        # (s0+s2), then B (s1+s3); single sync HWDGE queue (HBM-bound anyway)
        for s in range(4):
            xsl = xpool.tile([128, CCH, 512], bf16, name=f"xslab{s}")
            xs.append(xsl)

        def slab_dma(s, c0, c1, eng=None):
            (eng or nc.sync).dma_start(
                out=xs[s][:, c0:c1, :],
                in_=xkt_r[:, c0:c1, s * 512 : (s + 1) * 512],
            )

        for s in (0, 1, 2):
            slab_dma(s, 0, 4)
            slab_dma(s, 4, 8)
        mask_sb = consts.tile([128, 2 * N_SLOTS, QTILE], bf16)
        scal_sb = consts.tile([128, N_SLOTS], fp32)
        nc.sync.dma_start(out=scal_sb, in_=scald)
        nc.sync.dma_start(out=mask_sb, in_=maskd)
        # slab 3 in column halves: chunks 12-13 land before 14-15
        for q in range(2):
            nc.sync.dma_start(
                out=xs[3][:, :, q * 256 : (q + 1) * 256],
                in_=xkt_r[:, :, 3 * 512 + q * 256 : 3 * 512 + (q + 1) * 256],
            )

        # ---- Q projection (col-paired: slabs 0,1 -> psum halves) ----
        qT = kqpool.tile([128, NQ], bf16)   # Q^T duplicated in both halves

        def filler(n, tag):
            f_ps = psP.tile([128, 128], fp32, tag="proj", name=f"warmf_{tag}")
            for w in range(n):
                nc.tensor.matmul(
                    f_ps, lhsT=warm_src, rhs=warm_src,
                    start=(w == 0), stop=(w == n - 1),
                )

        def qproj():
            q_ps = psP.tile([128, 512], fp32, tag="proj", name="qps")
            for c in range(CCH):
                nc.tensor.matmul(
                    q_ps[0:64, :], lhsT=w_sb[:, c, D : 2 * D], rhs=xs[0][:, c, :],
                    start=(c == 0), stop=(c == CCH - 1),
                )
                nc.tensor.matmul(
                    q_ps[64:128, :], lhsT=w_sb[:, c, D : 2 * D], rhs=xs[1][:, c, :],
                    start=(c == 0), stop=(c == CCH - 1),
                )
            nc.scalar.copy(qT[0:64, 0:512], q_ps[0:64, :])
            nc.scalar.copy(qT[0:64, 512:1024], q_ps[64:128, :])
            # duplicate into partitions 64-127 (cross-partition -> DMA).
            # gpsimd queue: the sync HWDGE queue is FIFO and still busy
            # with the x slabs -- this copy must not wait behind them.
            nc.gpsimd.dma_start(out=qT[64:128, :], in_=qT[0:64, :])

        # ---- K projection stacks + transposes + attention slots ----
        # stack A: top = permuted cols 0-511 (chunks 0-3),
        #          bottom = cols 1024-1535 (chunks 8-11)   [slabs 0, 2]
        # stack B: top = 512-1023 (4-7), bottom = 1536-2047 (12-15) [1, 3]
        kstk = []
        vones = []
        o_done = []

        # slab for (stack, half): A=(s0 top, s2 bottom), B=(s1 top, s3 bottom)
        SLAB = {(0, 0): 0, (0, 1): 2, (1, 0): 1, (1, 1): 3}
        for si in range(2):
            kt = kqpool.tile([128, 512], bf16, name=f"kstk{si}")
            kstk.append(kt)
            vo = kqpool.tile([128, 8, D + 1], bf16, name=f"vones{si}")
            nc.vector.memset(vo[:, :, D : D + 1], 1.0)
            vones.append(vo)

        def kproj_half(si, half, q=None, cast_dve=False):
            """solo M=64 projection of one 512-col half into kstk[si].
            q selects a 256-col quarter (for the late B-bottom path)."""
            slab = xs[SLAB[(si, half)]]
            cs = slice(0, 512) if q is None else slice(q * 256, (q + 1) * 256)
            k_ps = psP.tile([128, 512], fp32, tag="proj",
                            name=f"kps{si}_{half}_{q}")
            hs = slice(64 * half, 64 * half + 64)
            for c in range(CCH):
                nc.tensor.matmul(
                    k_ps[hs, cs], lhsT=w_sb[:, c, 0:D], rhs=slab[:, c, cs],
                    start=(c == 0), stop=(c == CCH - 1),
                )
            if cast_dve:
                nc.vector.tensor_copy(kstk[si][hs, cs], k_ps[hs, cs])
            else:
                nc.scalar.copy(kstk[si][hs, cs], k_ps[hs, cs])

        def transp_half(si, half, only_p0=None):
            """V natural (+ones) for the 4 chunks of one half of stack si."""
            vo = vones[si]
            for p0 in ((0, 1) if only_p0 is None else (only_p0,)):
                pt2 = psP.tile(
                    [128, 128], bf16, tag="proj", name=f"tp{si}_{half}_{p0}"
                )
                for dk in range(2):
                    within = p0 * 2 + dk
                    nc.tensor.transpose(
                        pt2[:, dk * 64 : (dk + 1) * 64],
                        in_=kstk[si][64 * half : 64 * half + 64,
                                     within * CHUNK : (within + 1) * CHUNK],
                        identity=ident[64 * half : 64 * half + 64,
                                       64 * half : 64 * half + 64],
                    )
                w0 = half * 4 + p0 * 2
                nc.vector.tensor_copy(vo[:, w0 : w0 + 2, 0:D], pt2)

        def lhsT_of(c):
            si, half, within = _chunk_stack(c)
            return kstk[si][64 * half : 64 * half + 64,
                            within * CHUNK : (within + 1) * CHUNK]

        def vones_of(c):
            si, half, within = _chunk_stack(c)
            return vones[si][:, half * 4 + within, :]

        o_tiles = {}
        pt_tiles = {}

        def sexp_group(j, g):
            """S^T matmuls + exp (+ masks) for group g of slot j."""
            gch, kind = _slot_groups_def(j)[g]
            s_ps = psS.tile([128, GROUP * QTILE], fp32, tag="s",
                            name=f"sps{j}_{g}")
            order = (0, 2, 1, 3) if kind == "mixed" else (0, 1, 2, 3)
            for sl in order:
                cc = gch[sl]
                half = _chunk_stack(cc)[1]
                nc.tensor.matmul(
                    s_ps[:, sl * QTILE : (sl + 1) * QTILE],
                    lhsT=lhsT_of(cc),
                    rhs=qT[64 * half : 64 * half + 64,
                           j * QTILE : (j + 1) * QTILE],
                    start=True, stop=True,
                )
            pt = ptpool.tile([128, GROUP * QTILE], bf16, tag="pt", name=f"pt{j}_{g}")
            nc.scalar.activation(
                pt, s_ps, mybir.ActivationFunctionType.Exp, scale=SCALE
            )
            if kind == "mixed":
                nc.vector.tensor_mul(
                    pt[:, 0 : 2 * QTILE], pt[:, 0 : 2 * QTILE],
                    mask_sb[:, 2 * j : 2 * j + 2, :].rearrange("p g q -> p (g q)"),
                )
                nc.vector.tensor_scalar_mul(
                    pt[:, 2 * QTILE :], pt[:, 2 * QTILE :], scal_sb[:, j : j + 1]
                )
            elif kind == "own_diag":
                nc.vector.tensor_mul(
                    pt[:, 2 * QTILE :], pt[:, 2 * QTILE :],
                    mask_sb[:, 2 * j : 2 * j + 2, :].rearrange("p g q -> p (g q)"),
                )
            elif kind == "oth_tail":
                nc.vector.tensor_scalar_mul(
                    pt[:, 2 * QTILE :], pt[:, 2 * QTILE :], scal_sb[:, j : j + 1]
                )
            pt_tiles[(j, g)] = pt

        def pv_groups(j, glist):
            """PV accumulation for the given groups of slot j; finalizes
            (copy + DMA out) when the last group is included."""
            gdefs = _slot_groups_def(j)
            ngroups = len(gdefs)
            nch = ngroups * GROUP
            if j in o_tiles:
                o_ps = o_tiles[j]
            else:
                o_ps = psO.tile([D + 1, QTILE], fp32, tag="o", name=f"ops{j}")
                o_tiles[j] = o_ps
            for g in glist:
                gch, _ = gdefs[g]
                pt = pt_tiles.pop((j, g))
                for sl, cc in enumerate(gch):
                    k_abs = g * GROUP + sl
                    nc.tensor.matmul(
                        o_ps, lhsT=vones_of(cc),
                        rhs=pt[:, sl * QTILE : (sl + 1) * QTILE],
                        start=(k_abs == 0), stop=(k_abs == nch - 1),
                    )
            if glist[-1] == ngroups - 1:
                o_sb = opool.tile([D + 1, QTILE], fp32, name=f"osb{j}")
                nc.vector.tensor_copy(o_sb, o_ps)
                nc.gpsimd.dma_start(
                    out=out[:, j * QTILE : (j + 1) * QTILE], in_=o_sb
                )

        # emission order follows slab arrival: s0, s1, s2, s3
        kproj_half(0, 0)   # A-top    <- s0
        qproj()            # needs s0+s1
        kproj_half(1, 0)   # B-top    <- s1 (fills the s2 wait)
        # own-only S^T groups: need only A-top/B-top + qT -> exp starts early
        sexp_group(1, 0)   # {0,1,2,3}
        sexp_group(2, 0)
        sexp_group(3, 0)
        sexp_group(3, 2)   # {4,5,6,7} (B-top)
        transp_half(0, 0)
        transp_half(1, 0)
        kproj_half(0, 1, cast_dve=True)   # A-bottom <- s2
        sexp_group(0, 0)   # {0,1,8,9}
        sexp_group(1, 1)   # {8..11}
        sexp_group(2, 1)
        sexp_group(3, 1)
        transp_half(0, 1)
        kproj_half(1, 1, q=0, cast_dve=True)   # B-bottom chunks 12,13
        sexp_group(2, 2)        # {4,5,12,13}
        transp_half(1, 1, only_p0=0)
        kproj_half(1, 1, q=1, cast_dve=True)   # chunks 14,15
        sexp_group(3, 3)        # {12..15}
        transp_half(1, 1, only_p0=1)
        pv_groups(0, [0])
        pv_groups(1, [0, 1])
        pv_groups(2, [0, 1, 2])
        pv_groups(3, [0, 1, 2, 3])

    nc.compile()
    return nc


_NC_CACHE = None


def _get_nc():
    global _NC_CACHE
    if _NC_CACHE is None:
        _NC_CACHE = _build_graph()
    return _NC_CACHE


def _perm_tiles(r):
    """permuted 256-col tile order: own tiles (2j+r) first, then others."""
    own = [2 * j + r for j in range(N_SLOTS)]
    oth = [2 * j + (1 - r) for j in range(N_SLOTS)]
    return own + oth


def _host_prep(x, W_Q, W_K):
    in_maps = []
    wkq = np.concatenate([W_K.T, W_Q.T], axis=1).astype(BF16)  # [1024, 128]
    pchunk = np.arange(CHUNK)
    f = np.arange(QTILE)
    for i in range(N_CORES):
        b, r = i % B, i // B
        perm = _perm_tiles(r)
        xt = x[b].T.astype(BF16)  # [1024, 2048]
        cols = np.concatenate(
            [np.arange(QTILE * p, QTILE * p + QTILE) for p in perm]
        )
        xkt = np.ascontiguousarray(xt[:, cols])
        # mask[p, 2j+h, f]: causal mask of own diag chunks (2j, 2j+1);
        # scal[p, j]: 0/1 multiplier for the other-side tail chunks
        m = np.zeros((CHUNK, 2 * N_SLOTS, QTILE), dtype=np.float32)
        sc = np.zeros((CHUNK, N_SLOTS), dtype=np.float32)
        for j in range(N_SLOTS):
            t_abs = QTILE * (2 * j + r) + f[None, :]
            for h in range(2):
                c = 2 * j + h               # own chunk -> tile 2j+r
                s_abs = QTILE * (2 * j + r) + CHUNK * h + pchunk[:, None]
                m[:, 2 * j + h, :] = (s_abs <= t_abs)
            # oth chunks 8+2j, 8+2j+1 -> original tile 2j+(1-r):
            # r=1 -> tile 2j < own tile 2j+1: fully valid (1.0)
            # r=0 -> tile 2j+1 > own tile 2j: fully masked (0.0)
            sc[:, j] = float(r)
        in_maps.append(
            {
                "xkt": xkt,
                "wkq": wkq,
                "mask": np.ascontiguousarray(m.astype(BF16)),
                "scal": np.ascontiguousarray(sc.astype(np.float32)),
            }
        )
    return in_maps


def _ensure_ntff_hook():
    """Install the antenv.axon_hooks shim so trace=True works under axon."""
    import types

    try:
        from antenv.axon_hooks import get_axon_ntff_profile_hook  # noqa: F401

        return
    except ImportError:
        pass
    import antenv

    mod = types.ModuleType("antenv.axon_hooks")
    mod._hook = None

    def set_axon_ntff_profile_hook(h):
        mod._hook = h

    def get_axon_ntff_profile_hook():
        return mod._hook

    mod.set_axon_ntff_profile_hook = set_axon_ntff_profile_hook
    mod.get_axon_ntff_profile_hook = get_axon_ntff_profile_hook
    sys.modules["antenv.axon_hooks"] = mod
    antenv.axon_hooks = mod
    try:
        from trn_agent_boot.trn_boot import _ntff_profile_via_ctypes

        hook = _ntff_profile_via_ctypes("/opt/axon/libaxon_pjrt.so")
        if hook is not None:
            set_axon_ntff_profile_hook(hook)
    except Exception as e:  # degrade to no tracing
        print(f"ntff hook install failed: {e}")


def kernel(x, W_Q, W_K, W_V=None, **_unused):
    global LAST_RESULTS
    if TRACE:
        _ensure_ntff_hook()
    x = np.asarray(x, dtype=np.float32)
    W_Q = np.asarray(W_Q, dtype=np.float32)
    W_K = np.asarray(W_K, dtype=np.float32)

    from concourse.bass_utils import run_bass_kernel_spmd

    nc = _get_nc()
    in_maps = _host_prep(x, W_Q, W_K)
    res = run_bass_kernel_spmd(
        nc,
        in_maps,
        core_ids=list(range(N_CORES)),
        trace=TRACE,
        trace_cores=TRACE_CORES,
    )
    LAST_RESULTS = res

    y = np.empty((B, T, D), dtype=np.float32)
    for i in range(N_CORES):
        b, r = i % B, i // B
        ot = res.results[i]["out"]  # [65, 1024]
        o = ot[0:D, :] / ot[D : D + 1, :]
        for j in range(N_SLOTS):
            t0 = QTILE * (2 * j + r)
            y[b, t0 : t0 + QTILE, :] = o[:, j * QTILE : (j + 1) * QTILE].T
    return y


# revision 49
# speedup vs baseline: 1.1384x; 1.1384x over previous
"""Single-head causal attention (V=K source bug) on 8 trn2 NeuronCores.

Problem: x[4,2048,1024], W_Q/W_K/W_V[64,1024] (W_V unused by reference).
  Q = x @ W_Q.T ; K = x @ W_K.T ; V = K (reference bug)
  out = softmax(mask(Q K^T / sqrt(1024))) @ V      -> [4,2048,64]

Sharding: 2 cores per batch (core i: batch = i % 4, role r = i // 4).
Each batch's 8 query tiles of 256 rows split by parity (r=0 even, r=1 odd).
ONE SPMD graph for all 8 cores. Per-core differences are folded into DATA:

 * x^T is sent column-PERMUTED, own query tiles first:
     positions 0..3 = own tiles (2j+r), positions 4..7 = other tiles.
   So the Q projection reads compile-time columns [0,1024); causality over
   the permuted key order is encoded in per-core 0/1 masks.
 * slot j (own tile 2j+r, query rows 256 of it) attends own chunks
   [0..2j+1] and other chunks [8..8+2j+1] (uniform r=1 shape; r=0 masks
   the over-provisioned tail) -> 4j+4 key chunks of 128.

Device pipeline: warmup matmuls (HAM) -> col-paired projections
(Q pair, K stack A = cols 0-511|1024-1535, K stack B) -> PE transposes for
V natural -> per-slot: row-packed S^T pairs (own chunk on array rows 0-63,
other chunk on rows 64-127, concurrent), exp on ACT (scale folded), mask
mul on the final group, PV matmul with a ones-column producing the softmax
denominator in row 64. Host normalizes + transposes the [65,1024] output.
No collectives (latency floor >> kernel time).
"""

import os
import sys

sys.path.insert(0, "/opt/trn_rl_repo")

import numpy as np
import ml_dtypes

BF16 = ml_dtypes.bfloat16

B, T, C, D = 4, 2048, 1024, 64
N_CORES = 8
QTILE = 256          # query rows per slot
N_SLOTS = 4
CHUNK = 128          # key chunk
GROUP = 4            # chunks per exp group ([128, 4*256] psum tile)
SCALE = C ** -0.5
N_WARMUP = 80        # HAM warmup matmuls (cover the DMA wait before Q proj)

TRACE = False
TRACE_CORES = None
LAST_RESULTS = None


def _slot_groups_def(j):
    """Groups of 4 chunks for slot j with mask kind per group.
    Kinds: 'mixed' (slices 0-1 own diag MUL, 2-3 oth TS),
           'own_diag' (slices 2-3 MUL), 'oth_tail' (slices 2-3 TS),
           'plain'. Own chunks are 0..2j+1, other chunks 8..8+2j+1."""
    if j == 0:
        return [([0, 1, 8, 9], "mixed")]
    if j == 1:
        return [([0, 1, 2, 3], "own_diag"), ([8, 9, 10, 11], "oth_tail")]
    if j == 2:
        return [
            ([0, 1, 2, 3], "plain"),
            ([8, 9, 10, 11], "plain"),
            ([4, 5, 12, 13], "mixed"),
        ]
    return [
        ([0, 1, 2, 3], "plain"),
        ([8, 9, 10, 11], "plain"),
        ([4, 5, 6, 7], "own_diag"),
        ([12, 13, 14, 15], "oth_tail"),
    ]


def _chunk_stack(c):
    """abs permuted chunk c -> (stack_idx, half, within). Stack A covers
    permuted cols 0-511 (top) and 1024-1535 (bottom); B covers 512-1023
    (top) and 1536-2047 (bottom)."""
    pos = c // 2            # 256-col tile position 0..7
    if pos < 4:             # own side -> top halves
        return (pos // 2, 0, c % 4)
    else:                   # other side -> bottom halves
        return ((pos - 4) // 2, 1, c % 4)


def _build_graph():
    import concourse.bass as bass
    import concourse.mybir as mybir
    import concourse.tile as tile
    from concourse import bacc
    from concourse.masks import make_identity
    from contextlib import ExitStack

    fp32 = mybir.dt.float32
    bf16 = mybir.dt.bfloat16

    nc = bacc.Bacc(
        "TRN2",
        target_bir_lowering=False,
        debug=False,
        num_devices=N_CORES,
    )

    xkt = nc.dram_tensor("xkt", [C, T], bf16, kind="ExternalInput").ap()
    wkq = nc.dram_tensor("wkq", [C, 2 * D], bf16, kind="ExternalInput").ap()
    maskd = nc.dram_tensor(
        "mask", [CHUNK, 2 * N_SLOTS, QTILE], bf16, kind="ExternalInput"
    ).ap()
    scald = nc.dram_tensor(
        "scal", [CHUNK, N_SLOTS], fp32, kind="ExternalInput"
    ).ap()
    out = nc.dram_tensor(
        "out", [D + 1, N_SLOTS * QTILE], fp32, kind="ExternalOutput"
    ).ap()

    NQ = N_SLOTS * QTILE           # 1024 own query cols
    NCH = T // CHUNK               # 16 key chunks
    CCH = C // CHUNK               # 8 contraction chunks

    with tile.TileContext(nc) as tc, ExitStack() as ctx:
        consts = ctx.enter_context(tc.tile_pool(name="consts", bufs=1))
        xpool = ctx.enter_context(tc.tile_pool(name="xpool", bufs=1))
        kqpool = ctx.enter_context(tc.tile_pool(name="kqpool", bufs=1))
        ptpool = ctx.enter_context(tc.tile_pool(name="ptpool", bufs=10))
        opool = ctx.enter_context(tc.tile_pool(name="opool", bufs=2))
        psP = ctx.enter_context(tc.tile_pool(name="psP", bufs=2, space="PSUM"))
        psS = ctx.enter_context(tc.tile_pool(name="psS", bufs=2, space="PSUM"))
        psO = ctx.enter_context(tc.tile_pool(name="psO", bufs=2, space="PSUM"))

        # ---- constants ----
        # warmup matmuls on a memset tile: near-zero deps, start immediately
        warm_src = consts.tile([128, 128], bf16)
        nc.vector.memset(warm_src, 0.0)
        warm_ps = psP.tile([128, 128], fp32, tag="proj")
        for w in range(N_WARMUP):
            nc.tensor.matmul(
                warm_ps, lhsT=warm_src, rhs=warm_src,
                start=(w == 0), stop=(w == N_WARMUP - 1),
            )
        ident = consts.tile([128, 128], bf16)
        make_identity(nc, ident)
        warm = consts.tile([1, 1], fp32)
        nc.vector.memset(warm, 0.0)
        nc.scalar.activation(warm, warm, mybir.ActivationFunctionType.Exp)

        # ---- DMAs (slab order drives the pipeline) ----
        w_sb = consts.tile([128, CCH, 2 * D], bf16)
        nc.sync.dma_start(out=w_sb, in_=wkq.rearrange("(c p) d -> p c d", p=128))
        # xkt slabs: 4 x [128, CCH, 512] column slabs of the permuted x^T
        xs = []
        xkt_r = xkt.rearrange("(c p) t -> p c t", p=128)
        # interleave slab halves so Q (s0+s1) completes earliest, then A
        # (s0+s2), then B (s1+s3); single sync HWDGE queue (HBM-bound anyway)
        for s in range(4):
            xsl = xpool.tile([128, CCH, 512], bf16, name=f"xslab{s}")
            xs.append(xsl)

        def slab_dma(s, c0, c1, eng=None):
            (eng or nc.sync).dma_start(
                out=xs[s][:, c0:c1, :],
                in_=xkt_r[:, c0:c1, s * 512 : (s + 1) * 512],
            )

        for s in (0, 1, 2):
            slab_dma(s, 0, 4)
            slab_dma(s, 4, 8)
        mask_sb = consts.tile([128, 2 * N_SLOTS, QTILE], bf16)
        scal_sb = consts.tile([128, N_SLOTS], fp32)
        nc.sync.dma_start(out=scal_sb, in_=scald)
        nc.sync.dma_start(out=mask_sb, in_=maskd)
        # slab 3 in column halves: chunks 12-13 land before 14-15
        for q in range(2):
            nc.sync.dma_start(
                out=xs[3][:, :, q * 256 : (q + 1) * 256],
                in_=xkt_r[:, :, 3 * 512 + q * 256 : 3 * 512 + (q + 1) * 256],
            )

        # ---- Q projection (col-paired: slabs 0,1 -> psum halves) ----
        qT = kqpool.tile([128, NQ], bf16)   # Q^T duplicated in both halves

        def filler(n, tag):
            f_ps = psP.tile([128, 128], fp32, tag="proj", name=f"warmf_{tag}")
            for w in range(n):
                nc.tensor.matmul(
                    f_ps, lhsT=warm_src, rhs=warm_src,
                    start=(w == 0), stop=(w == n - 1),
                )

        def qproj():
            q_ps = psP.tile([128, 512], fp32, tag="proj", name="qps")
            for c in range(CCH):
                nc.tensor.matmul(
                    q_ps[0:64, :], lhsT=w_sb[:, c, D : 2 * D], rhs=xs[0][:, c, :],
                    start=(c == 0), stop=(c == CCH - 1),
                )
                nc.tensor.matmul(
                    q_ps[64:128, :], lhsT=w_sb[:, c, D : 2 * D], rhs=xs[1][:, c, :],
                    start=(c == 0), stop=(c == CCH - 1),
                )
            nc.scalar.copy(qT[0:64, 0:512], q_ps[0:64, :])
            nc.scalar.copy(qT[0:64, 512:1024], q_ps[64:128, :])
            # duplicate into partitions 64-127 (cross-partition -> DMA).
            # gpsimd queue: the sync HWDGE queue is FIFO and still busy
            # with the x slabs -- this copy must not wait behind them.
            nc.gpsimd.dma_start(out=qT[64:128, :], in_=qT[0:64, :])

        # ---- K projection stacks + transposes + attention slots ----
        # stack A: top = permuted cols 0-511 (chunks 0-3),
        #          bottom = cols 1024-1535 (chunks 8-11)   [slabs 0, 2]
        # stack B: top = 512-1023 (4-7), bottom = 1536-2047 (12-15) [1, 3]
        kstk = []
        vones = []
        o_done = []

        # slab for (stack, half): A=(s0 top, s2 bottom), B=(s1 top, s3 bottom)
        SLAB = {(0, 0): 0, (0, 1): 2, (1, 0): 1, (1, 1): 3}
        for si in range(2):
            kt = kqpool.tile([128, 512], bf16, name=f"kstk{si}")
            kstk.append(kt)
            vo = kqpool.tile([128, 8, D + 1], bf16, name=f"vones{si}")
            nc.vector.memset(vo[:, :, D : D + 1], 1.0)
            vones.append(vo)

        def kproj_half(si, half, q=None, cast_dve=False):
            """solo M=64 projection of one 512-col half into kstk[si].
            q selects a 256-col quarter (for the late B-bottom path)."""
            slab = xs[SLAB[(si, half)]]
            cs = slice(0, 512) if q is None else slice(q * 256, (q + 1) * 256)
            k_ps = psP.tile([128, 512], fp32, tag="proj",
                            name=f"kps{si}_{half}_{q}")
            hs = slice(64 * half, 64 * half + 64)
            for c in range(CCH):
                nc.tensor.matmul(
                    k_ps[hs, cs], lhsT=w_sb[:, c, 0:D], rhs=slab[:, c, cs],
                    start=(c == 0), stop=(c == CCH - 1),
                )
            if cast_dve:
                nc.vector.tensor_copy(kstk[si][hs, cs], k_ps[hs, cs])
            else:
                nc.scalar.copy(kstk[si][hs, cs], k_ps[hs, cs])

        def transp_half(si, half, only_p0=None):
            """V natural (+ones) for the 4 chunks of one half of stack si."""
            vo = vones[si]
            for p0 in ((0, 1) if only_p0 is None else (only_p0,)):
                pt2 = psP.tile(
                    [128, 128], bf16, tag="proj", name=f"tp{si}_{half}_{p0}"
                )
                for dk in range(2):
                    within = p0 * 2 + dk
                    nc.tensor.transpose(
                        pt2[:, dk * 64 : (dk + 1) * 64],
                        in_=kstk[si][64 * half : 64 * half + 64,
                                     within * CHUNK : (within + 1) * CHUNK],
                        identity=ident[64 * half : 64 * half + 64,
                                       64 * half : 64 * half + 64],
                    )
                w0 = half * 4 + p0 * 2
                nc.vector.tensor_copy(vo[:, w0 : w0 + 2, 0:D], pt2)

        def lhsT_of(c):
            si, half, within = _chunk_stack(c)
            return kstk[si][64 * half : 64 * half + 64,
                            within * CHUNK : (within + 1) * CHUNK]

        def vones_of(c):
            si, half, within = _chunk_stack(c)
            return vones[si][:, half * 4 + within, :]

        o_tiles = {}
        pt_tiles = {}

        def sexp_group(j, g):
            """S^T matmuls + exp (+ masks) for group g of slot j."""
            gch, kind = _slot_groups_def(j)[g]
            s_ps = psS.tile([128, GROUP * QTILE], fp32, tag="s",
                            name=f"sps{j}_{g}")
            order = (0, 2, 1, 3) if kind == "mixed" else (0, 1, 2, 3)
            for sl in order:
                cc = gch[sl]
                half = _chunk_stack(cc)[1]
                nc.tensor.matmul(
                    s_ps[:, sl * QTILE : (sl + 1) * QTILE],
                    lhsT=lhsT_of(cc),
                    rhs=qT[64 * half : 64 * half + 64,
                           j * QTILE : (j + 1) * QTILE],
                    start=True, stop=True,
                )
            pt = ptpool.tile([128, GROUP * QTILE], bf16, tag="pt", name=f"pt{j}_{g}")
            nc.scalar.activation(
                pt, s_ps, mybir.ActivationFunctionType.Exp, scale=SCALE
            )
            if kind == "mixed":
                nc.vector.tensor_mul(
                    pt[:, 0 : 2 * QTILE], pt[:, 0 : 2 * QTILE],
                    mask_sb[:, 2 * j : 2 * j + 2, :].rearrange("p g q -> p (g q)"),
                )
                nc.vector.tensor_scalar_mul(
                    pt[:, 2 * QTILE :], pt[:, 2 * QTILE :], scal_sb[:, j : j + 1]
                )
            elif kind == "own_diag":
                nc.vector.tensor_mul(
                    pt[:, 2 * QTILE :], pt[:, 2 * QTILE :],
                    mask_sb[:, 2 * j : 2 * j + 2, :].rearrange("p g q -> p (g q)"),
                )
            elif kind == "oth_tail":
                nc.vector.tensor_scalar_mul(
                    pt[:, 2 * QTILE :], pt[:, 2 * QTILE :], scal_sb[:, j : j + 1]
                )
            pt_tiles[(j, g)] = pt

        def pv_groups(j, glist):
            """PV accumulation for the given groups of slot j; finalizes
            (copy + DMA out) when the last group is included."""
            gdefs = _slot_groups_def(j)
            ngroups = len(gdefs)
            nch = ngroups * GROUP
            if j in o_tiles:
                o_ps = o_tiles[j]
            else:
                o_ps = psO.tile([D + 1, QTILE], fp32, tag="o", name=f"ops{j}")
                o_tiles[j] = o_ps
            for g in glist:
                gch, _ = gdefs[g]
                pt = pt_tiles.pop((j, g))
                for sl, cc in enumerate(gch):
                    k_abs = g * GROUP + sl
                    nc.tensor.matmul(
                        o_ps, lhsT=vones_of(cc),
                        rhs=pt[:, sl * QTILE : (sl + 1) * QTILE],
                        start=(k_abs == 0), stop=(k_abs == nch - 1),
                    )
            if glist[-1] == ngroups - 1:
                o_sb = opool.tile([D + 1, QTILE], fp32, name=f"osb{j}")
                nc.vector.tensor_copy(o_sb, o_ps)
                nc.gpsimd.dma_start(
                    out=out[:, j * QTILE : (j + 1) * QTILE], in_=o_sb
                )

        # emission order follows slab arrival: s0, s1, s2, s3
        kproj_half(0, 0)   # A-top    <- s0
        filler(12, "q")
        qproj()            # needs s0+s1
        kproj_half(1, 0)   # B-top    <- s1 (fills the s2 wait)
        # own-only S^T groups: need only A-top/B-top + qT -> exp starts early
        sexp_group(1, 0)   # {0,1,2,3}
        sexp_group(2, 0)
        sexp_group(3, 0)
        sexp_group(3, 2)   # {4,5,6,7} (B-top)
        transp_half(0, 0)
        transp_half(1, 0)
        filler(12, "ab")
        kproj_half(0, 1, cast_dve=True)   # A-bottom <- s2
        sexp_group(0, 0)   # {0,1,8,9}
        sexp_group(1, 1)   # {8..11}
        sexp_group(2, 1)
        sexp_group(3, 1)
        transp_half(0, 1)
        kproj_half(1, 1, q=0, cast_dve=True)   # B-bottom chunks 12,13
        sexp_group(2, 2)        # {4,5,12,13}
        transp_half(1, 1, only_p0=0)
        kproj_half(1, 1, q=1, cast_dve=True)   # chunks 14,15
        sexp_group(3, 3)        # {12..15}
        transp_half(1, 1, only_p0=1)
        pv_groups(0, [0])
        pv_groups(1, [0, 1])
        pv_groups(2, [0, 1, 2])
        pv_groups(3, [0, 1, 2, 3])

    nc.compile()
    return nc


_NC_CACHE = None


def _get_nc():
    global _NC_CACHE
    if _NC_CACHE is None:
        _NC_CACHE = _build_graph()
    return _NC_CACHE


def _perm_tiles(r):
    """permuted 256-col tile order: own tiles (2j+r) first, then others."""
    own = [2 * j + r for j in range(N_SLOTS)]
    oth = [2 * j + (1 - r) for j in range(N_SLOTS)]
    return own + oth


def _host_prep(x, W_Q, W_K):
    in_maps = []
    wkq = np.concatenate([W_K.T, W_Q.T], axis=1).astype(BF16)  # [1024, 128]
    pchunk = np.arange(CHUNK)
    f = np.arange(QTILE)
    for i in range(N_CORES):
        b, r = i % B, i // B
        perm = _perm_tiles(r)
        xt = x[b].T.astype(BF16)  # [1024, 2048]
        cols = np.concatenate(
            [np.arange(QTILE * p, QTILE * p + QTILE) for p in perm]
        )
        xkt = np.ascontiguousarray(xt[:, cols])
        # mask[p, 2j+h, f]: causal mask of own diag chunks (2j, 2j+1);
        # scal[p, j]: 0/1 multiplier for the other-side tail chunks
        m = np.zeros((CHUNK, 2 * N_SLOTS, QTILE), dtype=np.float32)
        sc = np.zeros((CHUNK, N_SLOTS), dtype=np.float32)
        for j in range(N_SLOTS):
            t_abs = QTILE * (2 * j + r) + f[None, :]
            for h in range(2):
                c = 2 * j + h               # own chunk -> tile 2j+r
                s_abs = QTILE * (2 * j + r) + CHUNK * h + pchunk[:, None]
                m[:, 2 * j + h, :] = (s_abs <= t_abs)
            # oth chunks 8+2j, 8+2j+1 -> original tile 2j+(1-r):
            # r=1 -> tile 2j < own tile 2j+1: fully valid (1.0)
            # r=0 -> tile 2j+1 > own tile 2j: fully masked (0.0)
            sc[:, j] = float(r)
        in_maps.append(
            {
                "xkt": xkt,
                "wkq": wkq,
                "mask": np.ascontiguousarray(m.astype(BF16)),
                "scal": np.ascontiguousarray(sc.astype(np.float32)),
            }
        )
    return in_maps


def _ensure_ntff_hook():
    """Install the antenv.axon_hooks shim so trace=True works under axon."""
    import types

    try:
        from antenv.axon_hooks import get_axon_ntff_profile_hook  # noqa: F401

        return
    except ImportError:
        pass
    import antenv

    mod = types.ModuleType("antenv.axon_hooks")
    mod._hook = None

    def set_axon_ntff_profile_hook(h):
        mod._hook = h

    def get_axon_ntff_profile_hook():
        return mod._hook

    mod.set_axon_ntff_profile_hook = set_axon_ntff_profile_hook
    mod.get_axon_ntff_profile_hook = get_axon_ntff_profile_hook
    sys.modules["antenv.axon_hooks"] = mod
    antenv.axon_hooks = mod
    try:
        from trn_agent_boot.trn_boot import _ntff_profile_via_ctypes

        hook = _ntff_profile_via_ctypes("/opt/axon/libaxon_pjrt.so")
        if hook is not None:
            set_axon_ntff_profile_hook(hook)
    except Exception as e:  # degrade to no tracing
        print(f"ntff hook install failed: {e}")


def kernel(x, W_Q, W_K, W_V=None, **_unused):
    global LAST_RESULTS
    if TRACE:
        _ensure_ntff_hook()
    x = np.asarray(x, dtype=np.float32)
    W_Q = np.asarray(W_Q, dtype=np.float32)
    W_K = np.asarray(W_K, dtype=np.float32)

    from concourse.bass_utils import run_bass_kernel_spmd

    nc = _get_nc()
    in_maps = _host_prep(x, W_Q, W_K)
    res = run_bass_kernel_spmd(
        nc,
        in_maps,
        core_ids=list(range(N_CORES)),
        trace=TRACE,
        trace_cores=TRACE_CORES,
    )
    LAST_RESULTS = res

    y = np.empty((B, T, D), dtype=np.float32)
    for i in range(N_CORES):
        b, r = i % B, i // B
        ot = res.results[i]["out"]  # [65, 1024]
        o = ot[0:D, :] / ot[D : D + 1, :]
        for j in range(N_SLOTS):
            t0 = QTILE * (2 * j + r)
            y[b, t0 : t0 + QTILE, :] = o[:, j * QTILE : (j + 1) * QTILE].T
    return y


# revision 50
# speedup vs baseline: 1.1672x; 1.0253x over previous
"""Single-head causal attention (V=K source bug) on 8 trn2 NeuronCores.

Problem: x[4,2048,1024], W_Q/W_K/W_V[64,1024] (W_V unused by reference).
  Q = x @ W_Q.T ; K = x @ W_K.T ; V = K (reference bug)
  out = softmax(mask(Q K^T / sqrt(1024))) @ V      -> [4,2048,64]

Sharding: 2 cores per batch (core i: batch = i % 4, role r = i // 4).
Each batch's 8 query tiles of 256 rows split by parity (r=0 even, r=1 odd).
ONE SPMD graph for all 8 cores. Per-core differences are folded into DATA:

 * x^T is sent column-PERMUTED, own query tiles first:
     positions 0..3 = own tiles (2j+r), positions 4..7 = other tiles.
   So the Q projection reads compile-time columns [0,1024); causality over
   the permuted key order is encoded in per-core 0/1 masks.
 * slot j (own tile 2j+r, query rows 256 of it) attends own chunks
   [0..2j+1] and other chunks [8..8+2j+1] (uniform r=1 shape; r=0 masks
   the over-provisioned tail) -> 4j+4 key chunks of 128.

Device pipeline (emission order ~= data-arrival order; one NEFF, no
collectives -- their latency floor exceeds the whole kernel):
 * HAM warmup + filler matmuls keep the PE at 2.4 GHz across DMA waits.
 * Projections as col-paired M=64+64 matmuls (Q pair; K stacks
   A = permuted cols 0-511 | 1024-1535, B = 512-1023 | 1536-2047),
   PSUM->SBUF casts on the idle ScalarE early / VectorE mid-chain.
 * Per 4-chunk group: S^T = K^T-chunk(stationary) x Q^T(moving), mixed
   groups row-packed (own chunk on array rows 0-63 concurrent with other
   chunk on rows 64-127); exp on ScalarE ([128,1024] PSUM->SBUF bf16,
   1/sqrt(C) folded into the activation scale; no max-subtraction --
   |scores| <= ~1 by construction). The serial ~11us ACT exp chain is
   the critical path; group order follows slab arrival so it never
   starves. Causal masks: elementwise MUL on own-diagonal chunks,
   scalar 0/1 MUL on the padded other-side chunks.
 * V natural (V=K) via PE transposes of K^T; PV matmuls use
   lhsT=[V|ones] so PSUM row 64 accumulates the softmax denominator.
 * Host divides by row 64 and transposes the [65,1024] outputs back.
"""

import os
import sys

sys.path.insert(0, "/opt/trn_rl_repo")

import numpy as np
import ml_dtypes

BF16 = ml_dtypes.bfloat16

B, T, C, D = 4, 2048, 1024, 64
N_CORES = 8
QTILE = 256          # query rows per slot
N_SLOTS = 4
CHUNK = 128          # key chunk
GROUP = 4            # chunks per exp group ([128, 4*256] psum tile)
SCALE = C ** -0.5
N_WARMUP = 80        # HAM warmup matmuls (cover the DMA wait before Q proj)

TRACE = False
TRACE_CORES = None
LAST_RESULTS = None


def _slot_groups_def(j):
    """Groups of 4 chunks for slot j with mask kind per group.
    Kinds: 'mixed' (slices 0-1 own diag MUL, 2-3 oth TS),
           'own_diag' (slices 2-3 MUL), 'oth_tail' (slices 2-3 TS),
           'plain'. Own chunks are 0..2j+1, other chunks 8..8+2j+1."""
    if j == 0:
        return [([0, 1, 8, 9], "mixed")]
    if j == 1:
        return [([0, 1, 2, 3], "own_diag"), ([8, 9, 10, 11], "oth_tail")]
    if j == 2:
        return [
            ([0, 1, 2, 3], "plain"),
            ([8, 9, 10, 11], "plain"),
            ([4, 5, 12, 13], "mixed"),
        ]
    return [
        ([0, 1, 2, 3], "plain"),
        ([8, 9, 10, 11], "plain"),
        ([4, 5, 6, 7], "own_diag"),
        ([12, 13, 14, 15], "oth_tail"),
    ]


def _chunk_stack(c):
    """abs permuted chunk c -> (stack_idx, half, within). Stack A covers
    permuted cols 0-511 (top) and 1024-1535 (bottom); B covers 512-1023
    (top) and 1536-2047 (bottom)."""
    pos = c // 2            # 256-col tile position 0..7
    if pos < 4:             # own side -> top halves
        return (pos // 2, 0, c % 4)
    else:                   # other side -> bottom halves
        return ((pos - 4) // 2, 1, c % 4)


def _build_graph():
    import concourse.bass as bass
    import concourse.mybir as mybir
    import concourse.tile as tile
    from concourse import bacc
    from concourse.masks import make_identity
    from contextlib import ExitStack

    fp32 = mybir.dt.float32
    bf16 = mybir.dt.bfloat16

    nc = bacc.Bacc(
        "TRN2",
        target_bir_lowering=False,
        debug=False,
        num_devices=N_CORES,
    )

    xkt = nc.dram_tensor("xkt", [C, T], bf16, kind="ExternalInput").ap()
    wkq = nc.dram_tensor("wkq", [C, 2 * D], bf16, kind="ExternalInput").ap()
    maskd = nc.dram_tensor(
        "mask", [CHUNK, 2 * N_SLOTS, QTILE], bf16, kind="ExternalInput"
    ).ap()
    scald = nc.dram_tensor(
        "scal", [CHUNK, N_SLOTS], fp32, kind="ExternalInput"
    ).ap()
    out = nc.dram_tensor(
        "out", [D + 1, N_SLOTS * QTILE], fp32, kind="ExternalOutput"
    ).ap()

    NQ = N_SLOTS * QTILE           # 1024 own query cols
    NCH = T // CHUNK               # 16 key chunks
    CCH = C // CHUNK               # 8 contraction chunks

    with tile.TileContext(nc) as tc, ExitStack() as ctx:
        consts = ctx.enter_context(tc.tile_pool(name="consts", bufs=1))
        xpool = ctx.enter_context(tc.tile_pool(name="xpool", bufs=1))
        kqpool = ctx.enter_context(tc.tile_pool(name="kqpool", bufs=1))
        ptpool = ctx.enter_context(tc.tile_pool(name="ptpool", bufs=10))
        opool = ctx.enter_context(tc.tile_pool(name="opool", bufs=2))
        psP = ctx.enter_context(tc.tile_pool(name="psP", bufs=2, space="PSUM"))
        psS = ctx.enter_context(tc.tile_pool(name="psS", bufs=2, space="PSUM"))
        psO = ctx.enter_context(tc.tile_pool(name="psO", bufs=2, space="PSUM"))

        # ---- constants ----
        # warmup matmuls on a memset tile: near-zero deps, start immediately
        warm_src = consts.tile([128, 128], bf16)
        nc.vector.memset(warm_src, 0.0)
        warm_ps = psP.tile([128, 128], fp32, tag="proj")
        for w in range(N_WARMUP):
            nc.tensor.matmul(
                warm_ps, lhsT=warm_src, rhs=warm_src,
                start=(w == 0), stop=(w == N_WARMUP - 1),
            )
        ident = consts.tile([128, 128], bf16)
        make_identity(nc, ident)
        warm = consts.tile([1, 1], fp32)
        nc.vector.memset(warm, 0.0)
        nc.scalar.activation(warm, warm, mybir.ActivationFunctionType.Exp)

        # ---- DMAs (slab order drives the pipeline) ----
        w_sb = consts.tile([128, CCH, 2 * D], bf16)
        nc.sync.dma_start(out=w_sb, in_=wkq.rearrange("(c p) d -> p c d", p=128))
        # xkt slabs: 4 x [128, CCH, 512] column slabs of the permuted x^T
        xs = []
        xkt_r = xkt.rearrange("(c p) t -> p c t", p=128)
        # interleave slab halves so Q (s0+s1) completes earliest, then A
        # (s0+s2), then B (s1+s3); single sync HWDGE queue (HBM-bound anyway)
        for s in range(4):
            xsl = xpool.tile([128, CCH, 512], bf16, name=f"xslab{s}")
            xs.append(xsl)

        def slab_dma(s, c0, c1, eng=None):
            (eng or nc.sync).dma_start(
                out=xs[s][:, c0:c1, :],
                in_=xkt_r[:, c0:c1, s * 512 : (s + 1) * 512],
            )

        for s in (0, 1, 2):
            slab_dma(s, 0, 4)
            slab_dma(s, 4, 8)
        mask_sb = consts.tile([128, 2 * N_SLOTS, QTILE], bf16)
        scal_sb = consts.tile([128, N_SLOTS], fp32)
        nc.sync.dma_start(out=scal_sb, in_=scald)
        nc.sync.dma_start(out=mask_sb, in_=maskd)
        # slab 3 in column halves: chunks 12-13 land before 14-15
        for q in range(2):
            nc.sync.dma_start(
                out=xs[3][:, :, q * 256 : (q + 1) * 256],
                in_=xkt_r[:, :, 3 * 512 + q * 256 : 3 * 512 + (q + 1) * 256],
            )

        # ---- Q projection (col-paired: slabs 0,1 -> psum halves) ----
        qT = kqpool.tile([128, NQ], bf16)   # Q^T duplicated in both halves

        def filler(n, tag):
            f_ps = psP.tile([128, 128], fp32, tag="proj", name=f"warmf_{tag}")
            for w in range(n):
                nc.tensor.matmul(
                    f_ps, lhsT=warm_src, rhs=warm_src,
                    start=(w == 0), stop=(w == n - 1),
                )

        def qproj():
            q_ps = psP.tile([128, 512], fp32, tag="proj", name="qps")
            for c in range(CCH):
                nc.tensor.matmul(
                    q_ps[0:64, :], lhsT=w_sb[:, c, D : 2 * D], rhs=xs[0][:, c, :],
                    start=(c == 0), stop=(c == CCH - 1),
                )
                nc.tensor.matmul(
                    q_ps[64:128, :], lhsT=w_sb[:, c, D : 2 * D], rhs=xs[1][:, c, :],
                    start=(c == 0), stop=(c == CCH - 1),
                )
            nc.scalar.copy(qT[0:64, 0:512], q_ps[0:64, :])
            nc.scalar.copy(qT[0:64, 512:1024], q_ps[64:128, :])
            # duplicate into partitions 64-127 (cross-partition -> DMA).
            # gpsimd queue: the sync HWDGE queue is FIFO and still busy
            # with the x slabs -- this copy must not wait behind them.
            nc.gpsimd.dma_start(out=qT[64:128, :], in_=qT[0:64, :])

        # ---- K projection stacks + transposes + attention slots ----
        # stack A: top = permuted cols 0-511 (chunks 0-3),
        #          bottom = cols 1024-1535 (chunks 8-11)   [slabs 0, 2]
        # stack B: top = 512-1023 (4-7), bottom = 1536-2047 (12-15) [1, 3]
        kstk = []
        vones = []
        o_done = []

        # slab for (stack, half): A=(s0 top, s2 bottom), B=(s1 top, s3 bottom)
        SLAB = {(0, 0): 0, (0, 1): 2, (1, 0): 1, (1, 1): 3}
        for si in range(2):
            kt = kqpool.tile([128, 512], bf16, name=f"kstk{si}")
            kstk.append(kt)
            vo = kqpool.tile([128, 8, D + 1], bf16, name=f"vones{si}")
            nc.vector.memset(vo[:, :, D : D + 1], 1.0)
            vones.append(vo)

        def kproj_half(si, half, q=None, cast_dve=False):
            """solo M=64 projection of one 512-col half into kstk[si].
            q selects a 256-col quarter (for the late B-bottom path)."""
            slab = xs[SLAB[(si, half)]]
            cs = slice(0, 512) if q is None else slice(q * 256, (q + 1) * 256)
            k_ps = psP.tile([128, 512], fp32, tag="proj",
                            name=f"kps{si}_{half}_{q}")
            hs = slice(64 * half, 64 * half + 64)
            for c in range(CCH):
                nc.tensor.matmul(
                    k_ps[hs, cs], lhsT=w_sb[:, c, 0:D], rhs=slab[:, c, cs],
                    start=(c == 0), stop=(c == CCH - 1),
                )
            if cast_dve:
                nc.vector.tensor_copy(kstk[si][hs, cs], k_ps[hs, cs])
            else:
                nc.scalar.copy(kstk[si][hs, cs], k_ps[hs, cs])

        def transp_half(si, half, only_p0=None):
            """V natural (+ones) for the 4 chunks of one half of stack si."""
            vo = vones[si]
            for p0 in ((0, 1) if only_p0 is None else (only_p0,)):
                pt2 = psP.tile(
                    [128, 128], bf16, tag="proj", name=f"tp{si}_{half}_{p0}"
                )
                for dk in range(2):
                    within = p0 * 2 + dk
                    nc.tensor.transpose(
                        pt2[:, dk * 64 : (dk + 1) * 64],
                        in_=kstk[si][64 * half : 64 * half + 64,
                                     within * CHUNK : (within + 1) * CHUNK],
                        identity=ident[64 * half : 64 * half + 64,
                                       64 * half : 64 * half + 64],
                    )
                w0 = half * 4 + p0 * 2
                nc.vector.tensor_copy(vo[:, w0 : w0 + 2, 0:D], pt2)

        def lhsT_of(c):
            si, half, within = _chunk_stack(c)
            return kstk[si][64 * half : 64 * half + 64,
                            within * CHUNK : (within + 1) * CHUNK]

        def vones_of(c):
            si, half, within = _chunk_stack(c)
            return vones[si][:, half * 4 + within, :]

        o_tiles = {}
        pt_tiles = {}

        def sexp_group(j, g):
            """S^T matmuls + exp (+ masks) for group g of slot j."""
            gch, kind = _slot_groups_def(j)[g]
            s_ps = psS.tile([128, GROUP * QTILE], fp32, tag="s",
                            name=f"sps{j}_{g}")
            order = (0, 2, 1, 3) if kind == "mixed" else (0, 1, 2, 3)
            for sl in order:
                cc = gch[sl]
                half = _chunk_stack(cc)[1]
                nc.tensor.matmul(
                    s_ps[:, sl * QTILE : (sl + 1) * QTILE],
                    lhsT=lhsT_of(cc),
                    rhs=qT[64 * half : 64 * half + 64,
                           j * QTILE : (j + 1) * QTILE],
                    start=True, stop=True,
                )
            pt = ptpool.tile([128, GROUP * QTILE], bf16, tag="pt", name=f"pt{j}_{g}")
            nc.scalar.activation(
                pt, s_ps, mybir.ActivationFunctionType.Exp, scale=SCALE
            )
            if kind == "mixed":
                nc.vector.tensor_mul(
                    pt[:, 0 : 2 * QTILE], pt[:, 0 : 2 * QTILE],
                    mask_sb[:, 2 * j : 2 * j + 2, :].rearrange("p g q -> p (g q)"),
                )
                nc.vector.tensor_scalar_mul(
                    pt[:, 2 * QTILE :], pt[:, 2 * QTILE :], scal_sb[:, j : j + 1]
                )
            elif kind == "own_diag":
                nc.vector.tensor_mul(
                    pt[:, 2 * QTILE :], pt[:, 2 * QTILE :],
                    mask_sb[:, 2 * j : 2 * j + 2, :].rearrange("p g q -> p (g q)"),
                )
            elif kind == "oth_tail":
                nc.vector.tensor_scalar_mul(
                    pt[:, 2 * QTILE :], pt[:, 2 * QTILE :], scal_sb[:, j : j + 1]
                )
            pt_tiles[(j, g)] = pt

        def pv_groups(j, glist):
            """PV accumulation for the given groups of slot j; finalizes
            (copy + DMA out) when the last group is included."""
            gdefs = _slot_groups_def(j)
            ngroups = len(gdefs)
            nch = ngroups * GROUP
            if j in o_tiles:
                o_ps = o_tiles[j]
            else:
                o_ps = psO.tile([D + 1, QTILE], fp32, tag="o", name=f"ops{j}")
                o_tiles[j] = o_ps
            for g in glist:
                gch, _ = gdefs[g]
                pt = pt_tiles.pop((j, g))
                for sl, cc in enumerate(gch):
                    k_abs = g * GROUP + sl
                    nc.tensor.matmul(
                        o_ps, lhsT=vones_of(cc),
                        rhs=pt[:, sl * QTILE : (sl + 1) * QTILE],
                        start=(k_abs == 0), stop=(k_abs == nch - 1),
                    )
            if glist[-1] == ngroups - 1:
                o_sb = opool.tile([D + 1, QTILE], fp32, name=f"osb{j}")
                nc.vector.tensor_copy(o_sb, o_ps)
                nc.gpsimd.dma_start(
                    out=out[:, j * QTILE : (j + 1) * QTILE], in_=o_sb
                )

        # emission order follows slab arrival: s0, s1, s2, s3
        kproj_half(0, 0)   # A-top    <- s0
        filler(12, "q")
        qproj()            # needs s0+s1
        kproj_half(1, 0)   # B-top    <- s1 (fills the s2 wait)
        # own-only S^T groups: need only A-top/B-top + qT -> exp starts early
        sexp_group(1, 0)   # {0,1,2,3}
        sexp_group(2, 0)
        sexp_group(3, 0)
        sexp_group(3, 2)   # {4,5,6,7} (B-top)
        transp_half(0, 0)
        transp_half(1, 0)
        filler(12, "ab")
        kproj_half(0, 1, cast_dve=True)   # A-bottom <- s2
        sexp_group(0, 0)   # {0,1,8,9}
        sexp_group(1, 1)   # {8..11}
        sexp_group(2, 1)
        sexp_group(3, 1)
        transp_half(0, 1)
        kproj_half(1, 1, q=0, cast_dve=True)   # B-bottom chunks 12,13
        sexp_group(2, 2)        # {4,5,12,13}
        transp_half(1, 1, only_p0=0)
        kproj_half(1, 1, q=1, cast_dve=True)   # chunks 14,15
        sexp_group(3, 3)        # {12..15}
        transp_half(1, 1, only_p0=1)
        pv_groups(0, [0])
        pv_groups(1, [0, 1])
        pv_groups(2, [0, 1, 2])
        pv_groups(3, [0, 1, 2, 3])

    nc.compile()
    return nc


_NC_CACHE = None


def _get_nc():
    global _NC_CACHE
    if _NC_CACHE is None:
        _NC_CACHE = _build_graph()
    return _NC_CACHE


def _perm_tiles(r):
    """permuted 256-col tile order: own tiles (2j+r) first, then others."""
    own = [2 * j + r for j in range(N_SLOTS)]
    oth = [2 * j + (1 - r) for j in range(N_SLOTS)]
    return own + oth


def _host_prep(x, W_Q, W_K):
    in_maps = []
    wkq = np.concatenate([W_K.T, W_Q.T], axis=1).astype(BF16)  # [1024, 128]
    pchunk = np.arange(CHUNK)
    f = np.arange(QTILE)
    for i in range(N_CORES):
        b, r = i % B, i // B
        perm = _perm_tiles(r)
        xt = x[b].T.astype(BF16)  # [1024, 2048]
        cols = np.concatenate(
            [np.arange(QTILE * p, QTILE * p + QTILE) for p in perm]
        )
        xkt = np.ascontiguousarray(xt[:, cols])
        # mask[p, 2j+h, f]: causal mask of own diag chunks (2j, 2j+1);
        # scal[p, j]: 0/1 multiplier for the other-side tail chunks
        m = np.zeros((CHUNK, 2 * N_SLOTS, QTILE), dtype=np.float32)
        sc = np.zeros((CHUNK, N_SLOTS), dtype=np.float32)
        for j in range(N_SLOTS):
            t_abs = QTILE * (2 * j + r) + f[None, :]
            for h in range(2):
                c = 2 * j + h               # own chunk -> tile 2j+r
                s_abs = QTILE * (2 * j + r) + CHUNK * h + pchunk[:, None]
                m[:, 2 * j + h, :] = (s_abs <= t_abs)
            # oth chunks 8+2j, 8+2j+1 -> original tile 2j+(1-r):
            # r=1 -> tile 2j < own tile 2j+1: fully valid (1.0)
            # r=0 -> tile 2j+1 > own tile 2j: fully masked (0.0)
            sc[:, j] = float(r)
        in_maps.append(
            {
                "xkt": xkt,
                "wkq": wkq,
                "mask": np.ascontiguousarray(m.astype(BF16)),
                "scal": np.ascontiguousarray(sc.astype(np.float32)),
            }
        )
    return in_maps


def _ensure_ntff_hook():
    """Install the antenv.axon_hooks shim so trace=True works under axon."""
    import types

    try:
        from antenv.axon_hooks import get_axon_ntff_profile_hook  # noqa: F401

        return
    except ImportError:
        pass
    import antenv

    mod = types.ModuleType("antenv.axon_hooks")
    mod._hook = None

    def set_axon_ntff_profile_hook(h):
        mod._hook = h

    def get_axon_ntff_profile_hook():
        return mod._hook

    mod.set_axon_ntff_profile_hook = set_axon_ntff_profile_hook
    mod.get_axon_ntff_profile_hook = get_axon_ntff_profile_hook
    sys.modules["antenv.axon_hooks"] = mod
    antenv.axon_hooks = mod
    try:
        from trn_agent_boot.trn_boot import _ntff_profile_via_ctypes

        hook = _ntff_profile_via_ctypes("/opt/axon/libaxon_pjrt.so")
        if hook is not None:
            set_axon_ntff_profile_hook(hook)
    except Exception as e:  # degrade to no tracing
        print(f"ntff hook install failed: {e}")


def kernel(x, W_Q, W_K, W_V=None, **_unused):
    global LAST_RESULTS
    if TRACE:
        _ensure_ntff_hook()
    x = np.asarray(x, dtype=np.float32)
    W_Q = np.asarray(W_Q, dtype=np.float32)
    W_K = np.asarray(W_K, dtype=np.float32)

    from concourse.bass_utils import run_bass_kernel_spmd

    nc = _get_nc()
    in_maps = _host_prep(x, W_Q, W_K)
    res = run_bass_kernel_spmd(
        nc,
        in_maps,
        core_ids=list(range(N_CORES)),
        trace=TRACE,
        trace_cores=TRACE_CORES,
    )
    LAST_RESULTS = res

    y = np.empty((B, T, D), dtype=np.float32)
    for i in range(N_CORES):
        b, r = i % B, i // B
        ot = res.results[i]["out"]  # [65, 1024]
        o = ot[0:D, :] / ot[D : D + 1, :]
        for j in range(N_SLOTS):
            t0 = QTILE * (2 * j + r)
            y[b, t0 : t0 + QTILE, :] = o[:, j * QTILE : (j + 1) * QTILE].T
    return y
